# revision 72
# baseline (speedup 1.0000x reference)
"""Trainium2 Bass kernel for nn_MultiHeadLinearAttention.

Full-input contract: kernel(**inputs) takes the unsharded numpy inputs and
returns the full output. Internally: data-parallel over batch across the 8
NeuronCores (B == 8, one batch element per core), no collectives.

Per-core math (S=2048, E=2048, H=16, d=128), bf16 matmuls + fp32 PSUM,
with the output projection in fp8e4 DoubleRow (2 k-tiles / 0.5 cyc-per-col):
  Pass A (per head):
    qT  = Wq[h]-stationary route            -> pqT [d, S] (kept for all heads)
    k   = xT-chunk-stationary route         -> pk  [S, d] (natural, transient)
    phi(x) = elu(x)+1 = min(exp(x), 1 + relu(x))   (exact identity)
    Gram trick: G[d', d] = sum_s xh[s,d'] pk[s,d]  (16 accum. matmuls)
                kv = G^T @ Wv[h]  (one matmul; never materializes v)
    ksum[d]  = sum_s pk[s,d]/SC  (ones column scaled by 1/SC)
  Pass B (per s-chunk sc), software-pipelined with the Wo stream:
    per head: nd[s,129] = pqT-chunk^T @ [kv|ksum]; inv = SC/den
    ctx pair: c_hi = fp8(num*inv), c_lo = fp8(num*inv - c_hi), interleaved
    as (hi,lo) byte pairs so one bf16 xbar-transpose moves both.
    out[sc,:] = sum_h sum_t DR-matmul(term_t) * 1/(SC*SW), where the three
    fp8 DoubleRow terms are (c_hi,W_hi), (c_lo,W_hi), (c_hi,W_lo) and
    W_hi+W_lo is the host-side fp8 split of Wo*SW.
Host does: x transpose + bf16/fp8 casts + weight packing + bias add + gather.
"""

import numpy as np
import ml_dtypes

import concourse.bass as bass
import concourse.mybir as mybir
import concourse.tile as tile
from concourse import bacc
from concourse.bass_utils import run_bass_kernel_spmd

S = 2048
E = 2048
H = 16
D = 128
N_CORES = 8
NCH = S // 128  # 16 s-chunks

SC = 32.0   # ctx scale into fp8
SW = 64.0   # Wo scale into fp8
OUT_SCALE = 1.0 / (SC * SW)

F32 = mybir.dt.float32
BF16 = mybir.dt.bfloat16
FP8 = mybir.dt.float8e4
AF = mybir.ActivationFunctionType
ALU = mybir.AluOpType
DR = mybir.MatmulPerfMode.DoubleRow

_CACHED = {}


def build_module():
    nc = bacc.Bacc("TRN2", target_bir_lowering=False, debug=False,
                   num_devices=N_CORES)

    xT = nc.dram_tensor("xT", [E, S], BF16, kind="ExternalInput")
    xn = nc.dram_tensor("xn", [H, 128, NCH * 128], BF16,
                        kind="ExternalInput")
    wq = nc.dram_tensor("wq", [D, H * D], BF16, kind="ExternalInput")
    wkv = nc.dram_tensor("wkv", [D, H * 2 * D], BF16, kind="ExternalInput")
    wo_hi = nc.dram_tensor("wo_hi", [D, H * E], FP8, kind="ExternalInput")
    wo_lo = nc.dram_tensor("wo_lo", [D, H * E], FP8, kind="ExternalInput")
    out = nc.dram_tensor("out", [S, E], BF16, kind="ExternalOutput")

    with tile.TileContext(nc) as tc:
        with (
            tc.tile_pool(name="const", bufs=1) as const,
            tc.tile_pool(name="work", bufs=2) as work,
            tc.tile_pool(name="psum", bufs=2, space="PSUM") as psum,
        ):
            # split the startup DMAs so head 0's slices land first
            wq_sb = const.tile([128, H * D], BF16)
            nc.sync.dma_start(out=wq_sb[:, 0:128], in_=wq[:, 0:128])
            wkv_sb = const.tile([128, H * 2 * D], BF16)
            nc.sync.dma_start(out=wkv_sb[:, 0:256], in_=wkv[:, 0:256])
            ones_col = const.tile([128, 1], BF16)
            nc.vector.memset(ones_col[:], 1.0 / SC)
            warm = const.tile([128, 1], F32)
            nc.vector.memset(warm[:], 0.0)
            nc.scalar.activation(warm[:], warm[:], AF.Exp)
            pqT = const.tile([128, H * S], BF16)      # all heads
            kv_all = const.tile([128, H * 129], BF16)  # all heads [kv|ksum]

            ctxT_tiles = {}
            ctx_tiles = {}

            def emit_nd_chunk(sc, h):
                ctx_sc = ctx_tiles[sc]
                pair8 = ctx_sc[:].bitcast(FP8).rearrange(
                    "p hh (j two) -> p hh j two", two=2)
                ndp = psum.tile([128, 129], F32, tag="g", bufs=3)
                nc.tensor.matmul(
                    ndp[:],
                    pqT[:, h * S + sc * 128:h * S + (sc + 1) * 128],
                    kv_all[:, h * 129:(h + 1) * 129],
                    start=True, stop=True)
                inv = work.tile([128, 1], F32, tag="inv", bufs=4)
                nc.vector.reciprocal(inv[:], ndp[:, 128:129])
                nc.scalar.activation(pair8[:, h, :, 0], ndp[:, 0:128],
                                     AF.Copy, scale=inv[:, 0:1])
                nc.vector.scalar_tensor_tensor(
                    pair8[:, h, :, 1], ndp[:, 0:128], inv[:, 0:1],
                    pair8[:, h, :, 0], ALU.mult, ALU.subtract)

            xh2_pending = []
            q2_done = set()

            # deferred q pieces: (piece_idx, col_start, col_len)
            Q2_PIECES = [(0, 256, 256), (1, 512, 512),
                         (2, 1024, 512), (3, 1536, 512)]
            Q2_ORDER = (1, 2, 3)

            def prefetch_q2(h, piece):
                _, c0, cl = Q2_PIECES[piece]
                xh2 = work.tile([128, 512], BF16, tag="xh2", bufs=4)
                nc.sync.dma_start(
                    out=xh2[:, 0:cl],
                    in_=xT[h * 128:(h + 1) * 128, c0:c0 + cl])
                xh2_pending.append(xh2)

            def emit_q2_half(h, piece):
                _, c0, cl = Q2_PIECES[piece]
                xh2 = xh2_pending.pop(0)
                q2p = psum.tile([128, 512], F32, tag="kv", bufs=1)
                nc.tensor.matmul(
                    q2p[:, 0:cl], wq_sb[:, h * 128:(h + 1) * 128],
                    xh2[:, 0:cl], start=True, stop=True)
                base = h * S + c0
                e2 = work.tile([128, 512], BF16, tag="e", bufs=3)
                nc.scalar.activation(e2[:, 0:cl], q2p[:, 0:cl], AF.Exp)
                t2 = work.tile([128, 512], BF16, tag="t", bufs=2)
                nc.vector.tensor_scalar(t2[:, 0:cl], q2p[:, 0:cl], 0.0, 1.0,
                                        ALU.max, ALU.add)
                nc.vector.tensor_tensor(pqT[:, base:base + cl],
                                        e2[:, 0:cl], t2[:, 0:cl], ALU.min)

            def emit_transpose(sc):
                ctx_sc = ctx_tiles.pop(sc)
                ctxT_sc = work.tile([128, H, 128], BF16, tag="ctxT", bufs=2)
                nc.sync.dma_start(
                    out=ctxT_sc[:],
                    in_=ctx_sc[:].rearrange("p h j -> p (h j)"),
                    transpose=True)
                ctxT_tiles[sc] = ctxT_sc

            # ---- pass-B step queue: nd heads + transpose, interleaved under
            # the PE-bound Wo stream so the scale chains never block PE.
            nd_steps = []

            def push_nd(sc):
                ctx_sc = work.tile([128, H, 128], BF16, tag="ctx", bufs=2)
                ctx_tiles[sc] = ctx_sc
                for h in range(H):
                    nd_steps.append((sc, h))
                nd_steps.append((sc, -1))  # transpose marker

            def drain_nd(n):
                for _ in range(n):
                    if not nd_steps:
                        return
                    sc, h = nd_steps.pop(0)
                    if h < 0:
                        emit_transpose(sc)
                    else:
                        # the nd read of pqT cols sc*128:(sc+1)*128 must be
                        # emitted after the deferred q2 write of that piece
                        while sc >= 4 and (h, sc // 4) not in q2_done:
                            drain_q2(1)
                        emit_nd_chunk(sc, h)

            def emit_accum_eop(ctxT_sc, sc, eop, hook=None):
                cT8 = ctxT_sc[:].bitcast(FP8).rearrange(
                    "p h (j two) -> p h j two", two=2)
                pa = psum.tile([128, 1024], F32, tag="pj", bufs=2)
                for hp in range(8):
                    if hook is not None:
                        hook(hp)
                    lhi = cT8[:, 2 * hp:2 * hp + 2, :, 0]
                    llo = cT8[:, 2 * hp:2 * hp + 2, :, 1]
                    for half in range(2):
                        eoq = eop * 2 + half
                        rhi = woq_hi[eoq][:, 2 * hp:2 * hp + 2, :]
                        rlo = woq_lo[eoq][:, 2 * hp:2 * hp + 2, :]
                        ps = pa[:, half * 512:(half + 1) * 512]
                        first = hp == 0
                        last = hp == 7
                        nc.tensor.matmul(ps, lhi, rhi, start=first,
                                         stop=False, perf_mode=DR)
                        nc.tensor.matmul(ps, llo, rhi, start=False,
                                         stop=False, perf_mode=DR)
                        nc.tensor.matmul(ps, lhi, rlo, start=False,
                                         stop=last, perf_mode=DR)
                if eop == 0:
                    out_t = work.tile([128, 2048], BF16, tag="outsb", bufs=2)
                    out_tiles[sc] = out_t
                    nc.scalar.activation(out_t[:, 0:1024], pa[:], AF.Copy,
                                         scale=OUT_SCALE)
                    if sc >= NCH - 2:
                        nc.sync.dma_start(
                            out=out[sc * 128:(sc + 1) * 128, 0:1024],
                            in_=out_t[:, 0:1024])
                else:
                    out_t = out_tiles.pop(sc)
                    if sc >= NCH - 2:
                        # split scale+store at 512 granularity so the last
                        # stores overlap the final matmuls / copies
                        for hh in range(2):
                            c0 = 1024 + hh * 512
                            nc.vector.tensor_scalar(
                                out_t[:, c0:c0 + 512], pa[:, hh * 512:
                                                          (hh + 1) * 512],
                                OUT_SCALE, None, ALU.mult)
                            nc.sync.dma_start(
                                out=out[sc * 128:(sc + 1) * 128,
                                        c0:c0 + 512],
                                in_=out_t[:, c0:c0 + 512])
                    else:
                        nc.scalar.activation(out_t[:, 1024:2048], pa[:],
                                             AF.Copy, scale=OUT_SCALE)
                        nc.sync.dma_start(
                            out=out[sc * 128:(sc + 1) * 128, :],
                            in_=out_t[:])

            wo_hi_v = wo_hi[:].rearrange("p (h q x) -> p h q x", q=4, x=512)
            wo_lo_v = wo_lo[:].rearrange("p (h q x) -> p h q x", q=4, x=512)
            woq_hi = []
            woq_lo = []
            out_tiles = {}

            # -------- Pass A: q/k projections, phi, Gram kv, per head -----
            xhT_pending = {}

            def prefetch_xhT(h):
                xhT = work.tile([128, S], BF16, tag="xhT", bufs=2)
                if h == 0:
                    nc.sync.dma_start(out=xhT[:, 0:512],
                                      in_=xT[0:128, 0:512])
                    nc.sync.dma_start(out=xhT[:, 512:2048],
                                      in_=xT[0:128, 512:2048])
                else:
                    nc.sync.dma_start(out=xhT[:],
                                      in_=xT[h * 128:(h + 1) * 128, :])
                xhT_pending[h] = xhT

            xn_pending = {}

            def prefetch_xn(h):
                xn_sb = work.tile([128, NCH, 128], BF16, tag="xn", bufs=2)
                # flat APs on both sides: 4KB contiguous descriptors (a 3-D
                # out AP forces 256B descriptors and doubles transfer time)
                nc.sync.dma_start(
                    out=xn_sb[:].rearrange("p c j -> p (c j)"),
                    in_=xn[h])
                xn_pending[h] = xn_sb

            prefetch_xhT(0)
            prefetch_xhT(1)
            for h in range(H):
                xhT = xhT_pending.pop(h)
                if h + 2 < H:
                    prefetch_xhT(h + 2)
                prefetch_xn(h)
                xn_sb = xn_pending.pop(h)
                if h == 0:
                    nc.sync.dma_start(out=wq_sb[:, 128:],
                                      in_=wq[:, 128:])
                    nc.sync.dma_start(out=wkv_sb[:, 256:],
                                      in_=wkv[:, 256:])

                # q (transposed layout) + phi -> pqT[h, 0:512]; the other
                # three 512-blocks are deferred into pass B (emit_q2_half)
                # to overlap their DVE work under the PE-bound Wo stream
                qp = psum.tile([128, 1024], F32, tag="pj", bufs=2)
                nc.tensor.matmul(
                    qp[:, 0:512], wq_sb[:, h * 128:(h + 1) * 128],
                    xhT[:, 0:512], start=True, stop=True)
                eq = work.tile([128, 512], BF16, tag="e", bufs=3)
                nc.scalar.activation(eq[:], qp[:, 0:512], AF.Exp)
                tq = work.tile([128, 512], BF16, tag="t", bufs=2)
                if h % 2 == 1:
                    nc.scalar.activation(tq[:], qp[:, 0:512], AF.Relu)
                    nc.vector.scalar_tensor_tensor(
                        pqT[:, h * S:h * S + 512], tq[:], 1.0, eq[:],
                        ALU.add, ALU.min)
                else:
                    nc.vector.tensor_scalar(tq[:], qp[:, 0:512], 0.0, 1.0,
                                            ALU.max, ALU.add)
                    nc.vector.tensor_tensor(pqT[:, h * S:h * S + 512],
                                            eq[:], tq[:], ALU.min)

                # k (natural layout) + phi -> pk
                # 'act' variant (exp + relu on ACT, fused add/min on DVE) vs
                # 'dve' variant (exp on ACT, add + min on DVE): alternate by
                # head parity so adjacent heads occupy different engines and
                # pipeline across the head loop.
                pk = work.tile([128, S], BF16, tag="pk", bufs=2)
                for j in range(2):
                    kp = psum.tile([128, 1024], F32, tag="pj", bufs=2)
                    for c in range(8):
                        sc = j * 8 + c
                        nc.tensor.matmul(
                            kp[:, c * 128:(c + 1) * 128],
                            xhT[:, sc * 128:(sc + 1) * 128],
                            wkv_sb[:, h * 256:h * 256 + 128],
                            start=True, stop=True)
                    dst = pk[:, j * 1024:(j + 1) * 1024]
                    e = work.tile([128, 1024], BF16, tag="e", bufs=3)
                    nc.scalar.activation(e[:], kp[:], AF.Exp)
                    if j == h % 2:
                        r = work.tile([128, 1024], BF16, tag="t", bufs=2)
                        nc.scalar.activation(r[:], kp[:], AF.Relu)
                        nc.vector.scalar_tensor_tensor(
                            dst, r[:], 1.0, e[:], ALU.add, ALU.min)
                    else:
                        t = work.tile([128, 1024], BF16, tag="t", bufs=2)
                        nc.vector.tensor_scalar(t[:], kp[:], 0.0, 1.0,
                                                ALU.max, ALU.add)
                        nc.vector.tensor_tensor(dst, e[:], t[:], ALU.min)

                if h == 10:
                    for eoq in range(4):
                        whi = work.tile([128, H, 512], FP8,
                                        tag=f"whi{eoq}", bufs=1)
                        nc.sync.dma_start(out=whi[:], in_=wo_hi_v[:, :, eoq, :])
                        woq_hi.append(whi)
                        wlo = work.tile([128, H, 512], FP8,
                                        tag=f"wlo{eoq}", bufs=1)
                        nc.sync.dma_start(out=wlo[:], in_=wo_lo_v[:, :, eoq, :])
                        woq_lo.append(wlo)

                # Gram G[d', d] + ksum -> kv_all[h]
                gp = psum.tile([128, 128], F32, tag="g", bufs=3)
                kvp = psum.tile([128, 129], F32, tag="kv", bufs=1)
                for c in range(NCH):
                    nc.tensor.matmul(gp[:], xn_sb[:, c, :],
                                     pk[:, c * 128:(c + 1) * 128],
                                     start=(c == 0), stop=(c == NCH - 1))
                    nc.tensor.matmul(kvp[:, 128:129],
                                     pk[:, c * 128:(c + 1) * 128],
                                     ones_col[:],
                                     start=(c == 0), stop=(c == NCH - 1))
                g_sb = work.tile([128, 128], BF16, tag="gsb")
                if h % 2 == 0:
                    nc.vector.tensor_copy(g_sb[:], gp[:])
                else:
                    nc.scalar.activation(g_sb[:], gp[:], AF.Copy)
                nc.tensor.matmul(kvp[:, 0:128], g_sb[:],
                                 wkv_sb[:, h * 256 + 128:h * 256 + 256],
                                 start=True, stop=True)
                if h % 2 == 0:
                    nc.scalar.activation(
                        kv_all[:, h * 129:(h + 1) * 129], kvp[:], AF.Copy)
                else:
                    nc.vector.tensor_copy(
                        kv_all[:, h * 129:(h + 1) * 129], kvp[:])
                if h == 0:
                    for esc in range(2):
                        ctxe = work.tile([128, H, 128], BF16,
                                         tag="ctx", bufs=2)
                        ctx_tiles[esc] = ctxe
                for esc in range(2):
                    emit_nd_chunk(esc, h)

            emit_transpose(0)
            emit_transpose(1)
            q2q = [(h, p) for p in Q2_ORDER for h in range(H)]
            q2q.reverse()  # pop() from the front

            q2pf = list(q2q)

            def drain_q2(n):
                for _ in range(n):
                    while q2pf and len(xh2_pending) < 4:
                        hq = q2pf.pop()
                        prefetch_q2(*hq)
                    if q2q:
                        h, half = q2q.pop()
                        emit_q2_half(h, half)
                        q2_done.add((h, half))

            def hook_e0(hp):
                # 2 nd steps every other hp: 16 heads + transpose over both eops
                if hp % 2 == 0:
                    drain_nd(2)
                if hp in (2, 5):
                    drain_q2(1)

            def hook_e1(hp):
                if hp % 2 == 0:
                    drain_nd(2)
                if hp in (1, 4, 7):
                    drain_q2(1)

            for sc in range(2, NCH + 2):
                if sc < NCH:
                    push_nd(sc)
                ctxT_sc = ctxT_tiles.pop(sc - 2)
                emit_accum_eop(ctxT_sc, sc - 2, 0, hook=hook_e0)
                emit_accum_eop(ctxT_sc, sc - 2, 1, hook=hook_e1)
                drain_nd(1)
                drain_q2(1)

    nc.compile()
    return nc


def get_module():
    if "nc" not in _CACHED:
        _CACHED["nc"] = build_module()
    return _CACHED["nc"]


def _bf16(a):
    return np.ascontiguousarray(a).astype(ml_dtypes.bfloat16)


def prepare_in_maps(inputs, Wq, Wk, Wv, Wo, bo):
    """Host-side shard + layout prep. Returns per-core input maps."""
    F8 = ml_dtypes.float8_e4m3
    wq_p = _bf16(np.transpose(np.asarray(Wq), (1, 0, 2)).reshape(D, H * D))
    wkv = np.concatenate([np.asarray(Wk), np.asarray(Wv)], axis=2)  # (H,d,2d)
    wkv_p = _bf16(np.transpose(wkv, (1, 0, 2)).reshape(D, H * 2 * D))
    wo_p = np.transpose(np.asarray(Wo).reshape(H, D, E),
                        (1, 0, 2)).reshape(D, H * E).astype(np.float32) * SW
    wo_hi = wo_p.astype(F8)
    wo_lo = (wo_p - wo_hi.astype(np.float32)).astype(F8)
    in_maps = []
    for b in range(N_CORES):
        xb = np.asarray(inputs[b])
        # xn packed per head: xn[h][p, c*128+j] = x[c*128+p, h*128+j]
        xn_p = _bf16(np.transpose(xb.reshape(NCH, 128, H, D),
                                  (2, 1, 0, 3)).reshape(H, 128, NCH * D))
        in_maps.append({"xT": _bf16(xb.T), "xn": xn_p,
                        "wq": wq_p, "wkv": wkv_p,
                        "wo_hi": wo_hi, "wo_lo": wo_lo})
    return in_maps


def kernel(inputs, Wq, Wk, Wv, Wo, bo):
    B = inputs.shape[0]
    assert B == N_CORES and inputs.shape[1:] == (S, E)
    nc = get_module()
    in_maps = prepare_in_maps(inputs, Wq, Wk, Wv, Wo, bo)
    res = run_bass_kernel_spmd(nc, in_maps, list(range(N_CORES)))
    outs = np.stack([res.results[b]["out"].astype(np.float32)
                     for b in range(N_CORES)], axis=0)
    return (outs + np.asarray(bo, dtype=np.float32)[None, None, :]).astype(
        np.float32)


# revision 78
# speedup vs baseline: 1.0172x; 1.0172x over previous
"""Trainium2 Bass kernel for nn_MultiHeadLinearAttention.

Full-input contract: kernel(**inputs) takes the unsharded numpy inputs and
returns the full output. Internally: data-parallel over batch across the 8
NeuronCores (B == 8, one batch element per core), no collectives.

Per-core math (S=2048, E=2048, H=16, d=128), bf16 matmuls + fp32 PSUM,
with the output projection in fp8e4 DoubleRow (2 k-tiles / 0.5 cyc-per-col):
  Pass A (per head):
    qT  = Wq[h]-stationary route            -> pqT [d, S] (kept for all heads)
    k   = xT-chunk-stationary route         -> pk  [S, d] (natural, transient)
    phi(x) = elu(x)+1 = min(exp(x), 1 + relu(x))   (exact identity)
    Gram trick: G[d', d] = sum_s xh[s,d'] pk[s,d]  (16 accum. matmuls)
                kv = G^T @ Wv[h]  (one matmul; never materializes v)
    ksum[d]  = sum_s pk[s,d]/SC  (ones column scaled by 1/SC)
  Pass B (per s-chunk sc), software-pipelined with the Wo stream:
    per head: nd[s,129] = pqT-chunk^T @ [kv|ksum]; inv = SC/den
    ctx pair: c_hi = fp8(num*inv), c_lo = fp8(num*inv - c_hi), interleaved
    as (hi,lo) byte pairs so one bf16 xbar-transpose moves both.
    out[sc,:] = sum_h sum_t DR-matmul(term_t) * 1/(SC*SW), where the three
    fp8 DoubleRow terms are (c_hi,W_hi), (c_lo,W_hi), (c_hi,W_lo) and
    W_hi+W_lo is the host-side fp8 split of Wo*SW.
Host does: x transpose + bf16/fp8 casts + weight packing + bias add + gather.
"""

import numpy as np
import ml_dtypes

import concourse.bass as bass
import concourse.mybir as mybir
import concourse.tile as tile
from concourse import bacc
from concourse.bass_utils import run_bass_kernel_spmd

S = 2048
E = 2048
H = 16
D = 128
N_CORES = 8
NCH = S // 128  # 16 s-chunks

SC = 32.0   # ctx scale into fp8
SW = 64.0   # Wo scale into fp8
OUT_SCALE = 1.0 / (SC * SW)

F32 = mybir.dt.float32
BF16 = mybir.dt.bfloat16
FP8 = mybir.dt.float8e4
AF = mybir.ActivationFunctionType
ALU = mybir.AluOpType
DR = mybir.MatmulPerfMode.DoubleRow

_CACHED = {}


def build_module():
    nc = bacc.Bacc("TRN2", target_bir_lowering=False, debug=False,
                   num_devices=N_CORES)

    xT = nc.dram_tensor("xT", [E, S], BF16, kind="ExternalInput")
    xn = nc.dram_tensor("xn", [H, 128, NCH * 128], BF16,
                        kind="ExternalInput")
    wq = nc.dram_tensor("wq", [D, H * D], BF16, kind="ExternalInput")
    wkv = nc.dram_tensor("wkv", [D, H * 2 * D], BF16, kind="ExternalInput")
    wo_hi = nc.dram_tensor("wo_hi", [D, H * E], FP8, kind="ExternalInput")
    wo_lo = nc.dram_tensor("wo_lo", [D, H * E], FP8, kind="ExternalInput")
    out = nc.dram_tensor("out", [S, E], BF16, kind="ExternalOutput")

    with tile.TileContext(nc) as tc:
        with (
            tc.tile_pool(name="const", bufs=1) as const,
            tc.tile_pool(name="work", bufs=2) as work,
            tc.tile_pool(name="psum", bufs=2, space="PSUM") as psum,
        ):
            # split the startup DMAs so head 0's slices land first
            wq_sb = const.tile([128, H * D], BF16)
            nc.sync.dma_start(out=wq_sb[:, 0:128], in_=wq[:, 0:128])
            wkv_sb = const.tile([128, H * 2 * D], BF16)
            nc.sync.dma_start(out=wkv_sb[:, 0:256], in_=wkv[:, 0:256])
            ones_col = const.tile([128, 1], BF16)
            nc.vector.memset(ones_col[:], 1.0 / SC)
            warm = const.tile([128, 1], F32)
            nc.vector.memset(warm[:], 0.0)
            nc.scalar.activation(warm[:], warm[:], AF.Exp)
            pqT = const.tile([128, H * S], BF16)      # all heads
            kv_all = const.tile([128, H * 129], BF16)  # all heads [kv|ksum]

            ctxT_tiles = {}
            ctx_tiles = {}

            def emit_nd_chunk(sc, h):
                ctx_sc = ctx_tiles[sc]
                pair8 = ctx_sc[:].bitcast(FP8).rearrange(
                    "p hh (j two) -> p hh j two", two=2)
                ndp = psum.tile([128, 129], F32, tag="g", bufs=3)
                nc.tensor.matmul(
                    ndp[:],
                    pqT[:, h * S + sc * 128:h * S + (sc + 1) * 128],
                    kv_all[:, h * 129:(h + 1) * 129],
                    start=True, stop=True)
                inv = work.tile([128, 1], F32, tag="inv", bufs=8)
                nc.vector.reciprocal(inv[:], ndp[:, 128:129])
                nc.scalar.activation(pair8[:, h, :, 0], ndp[:, 0:128],
                                     AF.Copy, scale=inv[:, 0:1])
                nc.vector.scalar_tensor_tensor(
                    pair8[:, h, :, 1], ndp[:, 0:128], inv[:, 0:1],
                    pair8[:, h, :, 0], ALU.mult, ALU.subtract)

            xh2_pending = []
            q2_done = set()

            # deferred q pieces: (piece_idx, col_start, col_len)
            Q2_PIECES = [(0, 256, 256), (1, 512, 512),
                         (2, 1024, 512), (3, 1536, 512)]
            Q2_ORDER = (1, 2, 3)

            def prefetch_q2(h, piece):
                _, c0, cl = Q2_PIECES[piece]
                xh2 = work.tile([128, 512], BF16, tag="xh2", bufs=4)
                nc.sync.dma_start(
                    out=xh2[:, 0:cl],
                    in_=xT[h * 128:(h + 1) * 128, c0:c0 + cl])
                xh2_pending.append(xh2)

            def emit_q2_half(h, piece):
                _, c0, cl = Q2_PIECES[piece]
                xh2 = xh2_pending.pop(0)
                q2p = psum.tile([128, 512], F32, tag="kv", bufs=1)
                nc.tensor.matmul(
                    q2p[:, 0:cl], wq_sb[:, h * 128:(h + 1) * 128],
                    xh2[:, 0:cl], start=True, stop=True)
                base = h * S + c0
                e2 = work.tile([128, 512], BF16, tag="e", bufs=3)
                nc.scalar.activation(e2[:, 0:cl], q2p[:, 0:cl], AF.Exp)
                t2 = work.tile([128, 512], BF16, tag="t", bufs=2)
                nc.vector.tensor_scalar(t2[:, 0:cl], q2p[:, 0:cl], 0.0, 1.0,
                                        ALU.max, ALU.add)
                nc.vector.tensor_tensor(pqT[:, base:base + cl],
                                        e2[:, 0:cl], t2[:, 0:cl], ALU.min)

            def emit_transpose(sc):
                ctx_sc = ctx_tiles.pop(sc)
                ctxT_sc = work.tile([128, H, 128], BF16, tag="ctxT", bufs=2)
                nc.sync.dma_start(
                    out=ctxT_sc[:],
                    in_=ctx_sc[:].rearrange("p h j -> p (h j)"),
                    transpose=True)
                ctxT_tiles[sc] = ctxT_sc

            # ---- pass-B step queue: nd heads + transpose, interleaved under
            # the PE-bound Wo stream so the scale chains never block PE.
            nd_steps = []

            def push_nd(sc):
                ctx_sc = work.tile([128, H, 128], BF16, tag="ctx", bufs=2)
                ctx_tiles[sc] = ctx_sc
                for h in range(H):
                    nd_steps.append((sc, h))
                nd_steps.append((sc, -1))  # transpose marker

            def drain_nd(n):
                for _ in range(n):
                    if not nd_steps:
                        return
                    sc, h = nd_steps.pop(0)
                    if h < 0:
                        emit_transpose(sc)
                    else:
                        # the nd read of pqT cols sc*128:(sc+1)*128 must be
                        # emitted after the deferred q2 write of that piece
                        while sc >= 4 and (h, sc // 4) not in q2_done:
                            drain_q2(1)
                        emit_nd_chunk(sc, h)

            def emit_accum_eop(ctxT_sc, sc, eop, hook=None):
                cT8 = ctxT_sc[:].bitcast(FP8).rearrange(
                    "p h (j two) -> p h j two", two=2)
                pa = psum.tile([128, 1024], F32, tag="pj", bufs=2)
                for hp in range(8):
                    if hook is not None:
                        hook(hp)
                    lhi = cT8[:, 2 * hp:2 * hp + 2, :, 0]
                    llo = cT8[:, 2 * hp:2 * hp + 2, :, 1]
                    for half in range(2):
                        eoq = eop * 2 + half
                        rhi = woq_hi[eoq][:, 2 * hp:2 * hp + 2, :]
                        rlo = woq_lo[eoq][:, 2 * hp:2 * hp + 2, :]
                        ps = pa[:, half * 512:(half + 1) * 512]
                        first = hp == 0
                        last = hp == 7
                        nc.tensor.matmul(ps, lhi, rhi, start=first,
                                         stop=False, perf_mode=DR)
                        nc.tensor.matmul(ps, llo, rhi, start=False,
                                         stop=False, perf_mode=DR)
                        nc.tensor.matmul(ps, lhi, rlo, start=False,
                                         stop=last, perf_mode=DR)
                if eop == 0:
                    out_t = work.tile([128, 2048], BF16, tag="outsb", bufs=2)
                    out_tiles[sc] = out_t
                    nc.scalar.activation(out_t[:, 0:1024], pa[:], AF.Copy,
                                         scale=OUT_SCALE)
                    if sc >= NCH - 2:
                        nc.sync.dma_start(
                            out=out[sc * 128:(sc + 1) * 128, 0:1024],
                            in_=out_t[:, 0:1024])
                else:
                    out_t = out_tiles.pop(sc)
                    if sc >= NCH - 2:
                        # split scale+store at 512 granularity so the last
                        # stores overlap the final matmuls / copies
                        for hh in range(2):
                            c0 = 1024 + hh * 512
                            nc.vector.tensor_scalar(
                                out_t[:, c0:c0 + 512], pa[:, hh * 512:
                                                          (hh + 1) * 512],
                                OUT_SCALE, None, ALU.mult)
                            nc.sync.dma_start(
                                out=out[sc * 128:(sc + 1) * 128,
                                        c0:c0 + 512],
                                in_=out_t[:, c0:c0 + 512])
                    else:
                        nc.scalar.activation(out_t[:, 1024:2048], pa[:],
                                             AF.Copy, scale=OUT_SCALE)
                        nc.sync.dma_start(
                            out=out[sc * 128:(sc + 1) * 128, :],
                            in_=out_t[:])

            wo_hi_v = wo_hi[:].rearrange("p (h q x) -> p h q x", q=4, x=512)
            wo_lo_v = wo_lo[:].rearrange("p (h q x) -> p h q x", q=4, x=512)
            woq_hi = []
            woq_lo = []
            out_tiles = {}

            # -------- Pass A: q/k projections, phi, Gram kv, per head -----
            xhT_pending = {}

            def prefetch_xhT(h):
                xhT = work.tile([128, S], BF16, tag="xhT", bufs=2)
                if h == 0:
                    nc.sync.dma_start(out=xhT[:, 0:512],
                                      in_=xT[0:128, 0:512])
                    nc.sync.dma_start(out=xhT[:, 512:2048],
                                      in_=xT[0:128, 512:2048])
                else:
                    nc.sync.dma_start(out=xhT[:],
                                      in_=xT[h * 128:(h + 1) * 128, :])
                xhT_pending[h] = xhT

            xn_pending = {}

            def prefetch_xn(h):
                xn_sb = work.tile([128, NCH, 128], BF16, tag="xn", bufs=2)
                # flat APs on both sides: 4KB contiguous descriptors (a 3-D
                # out AP forces 256B descriptors and doubles transfer time)
                nc.sync.dma_start(
                    out=xn_sb[:].rearrange("p c j -> p (c j)"),
                    in_=xn[h])
                xn_pending[h] = xn_sb

            prefetch_xhT(0)
            prefetch_xhT(1)
            for h in range(H):
                xhT = xhT_pending.pop(h)
                if h + 2 < H:
                    prefetch_xhT(h + 2)
                prefetch_xn(h)
                xn_sb = xn_pending.pop(h)
                if h == 0:
                    nc.sync.dma_start(out=wq_sb[:, 128:512],
                                      in_=wq[:, 128:512])
                    nc.sync.dma_start(out=wkv_sb[:, 256:1024],
                                      in_=wkv[:, 256:1024])
                elif h == 1:
                    nc.sync.dma_start(out=wq_sb[:, 512:1024],
                                      in_=wq[:, 512:1024])
                    nc.sync.dma_start(out=wkv_sb[:, 1024:2048],
                                      in_=wkv[:, 1024:2048])
                elif h == 2:
                    nc.sync.dma_start(out=wq_sb[:, 1024:],
                                      in_=wq[:, 1024:])
                    nc.sync.dma_start(out=wkv_sb[:, 2048:3072],
                                      in_=wkv[:, 2048:3072])
                elif h == 4:
                    nc.sync.dma_start(out=wkv_sb[:, 3072:],
                                      in_=wkv[:, 3072:])

                # q (transposed layout) + phi -> pqT[h, 0:512]; the other
                # three 512-blocks are deferred into pass B (emit_q2_half)
                # to overlap their DVE work under the PE-bound Wo stream
                qp = psum.tile([128, 1024], F32, tag="pj", bufs=2)
                nc.tensor.matmul(
                    qp[:, 0:512], wq_sb[:, h * 128:(h + 1) * 128],
                    xhT[:, 0:512], start=True, stop=True)
                eq = work.tile([128, 512], BF16, tag="e", bufs=3)
                nc.scalar.activation(eq[:], qp[:, 0:512], AF.Exp)
                tq = work.tile([128, 512], BF16, tag="t", bufs=2)
                if h % 2 == 1:
                    nc.scalar.activation(tq[:], qp[:, 0:512], AF.Relu)
                    nc.vector.scalar_tensor_tensor(
                        pqT[:, h * S:h * S + 512], tq[:], 1.0, eq[:],
                        ALU.add, ALU.min)
                else:
                    nc.vector.tensor_scalar(tq[:], qp[:, 0:512], 0.0, 1.0,
                                            ALU.max, ALU.add)
                    nc.vector.tensor_tensor(pqT[:, h * S:h * S + 512],
                                            eq[:], tq[:], ALU.min)

                # k (natural layout) + phi -> pk
                # 'act' variant (exp + relu on ACT, fused add/min on DVE) vs
                # 'dve' variant (exp on ACT, add + min on DVE): alternate by
                # head parity so adjacent heads occupy different engines and
                # pipeline across the head loop.
                pk = work.tile([128, S], BF16, tag="pk", bufs=2)
                for j in range(2):
                    kp = psum.tile([128, 1024], F32, tag="pj", bufs=2)
                    for c in range(8):
                        sc = j * 8 + c
                        nc.tensor.matmul(
                            kp[:, c * 128:(c + 1) * 128],
                            xhT[:, sc * 128:(sc + 1) * 128],
                            wkv_sb[:, h * 256:h * 256 + 128],
                            start=True, stop=True)
                    dst = pk[:, j * 1024:(j + 1) * 1024]
                    e = work.tile([128, 1024], BF16, tag="e", bufs=3)
                    nc.scalar.activation(e[:], kp[:], AF.Exp)
                    if j == h % 2:
                        r = work.tile([128, 1024], BF16, tag="t", bufs=2)
                        nc.scalar.activation(r[:], kp[:], AF.Relu)
                        nc.vector.scalar_tensor_tensor(
                            dst, r[:], 1.0, e[:], ALU.add, ALU.min)
                    else:
                        t = work.tile([128, 1024], BF16, tag="t", bufs=2)
                        nc.vector.tensor_scalar(t[:], kp[:], 0.0, 1.0,
                                                ALU.max, ALU.add)
                        nc.vector.tensor_tensor(dst, e[:], t[:], ALU.min)

                if h == 10:
                    for eoq in range(4):
                        whi = work.tile([128, H, 512], FP8,
                                        tag=f"whi{eoq}", bufs=1)
                        nc.sync.dma_start(out=whi[:], in_=wo_hi_v[:, :, eoq, :])
                        woq_hi.append(whi)
                        wlo = work.tile([128, H, 512], FP8,
                                        tag=f"wlo{eoq}", bufs=1)
                        nc.sync.dma_start(out=wlo[:], in_=wo_lo_v[:, :, eoq, :])
                        woq_lo.append(wlo)

                # Gram G[d', d] + ksum -> kv_all[h]
                gp = psum.tile([128, 128], F32, tag="g", bufs=3)
                kvp = psum.tile([128, 129], F32, tag="kv", bufs=1)
                for c in range(NCH):
                    nc.tensor.matmul(gp[:], xn_sb[:, c, :],
                                     pk[:, c * 128:(c + 1) * 128],
                                     start=(c == 0), stop=(c == NCH - 1))
                    nc.tensor.matmul(kvp[:, 128:129],
                                     pk[:, c * 128:(c + 1) * 128],
                                     ones_col[:],
                                     start=(c == 0), stop=(c == NCH - 1))
                g_sb = work.tile([128, 128], BF16, tag="gsb")
                nc.vector.tensor_copy(g_sb[:], gp[:])
                nc.tensor.matmul(kvp[:, 0:128], g_sb[:],
                                 wkv_sb[:, h * 256 + 128:h * 256 + 256],
                                 start=True, stop=True)
                nc.vector.tensor_copy(
                    kv_all[:, h * 129:(h + 1) * 129], kvp[:])
                if h == 0:
                    for esc in range(2):
                        ctxe = work.tile([128, H, 128], BF16,
                                         tag="ctx", bufs=2)
                        ctx_tiles[esc] = ctxe
                for esc in range(2):
                    emit_nd_chunk(esc, h)

            emit_transpose(0)
            emit_transpose(1)
            q2q = [(h, p) for p in Q2_ORDER for h in range(H)]
            q2q.reverse()  # pop() from the front

            q2pf = list(q2q)

            def drain_q2(n):
                for _ in range(n):
                    while q2pf and len(xh2_pending) < 4:
                        hq = q2pf.pop()
                        prefetch_q2(*hq)
                    if q2q:
                        h, half = q2q.pop()
                        emit_q2_half(h, half)
                        q2_done.add((h, half))

            def hook_e0(hp):
                # 2 nd steps every other hp: 16 heads + transpose over both eops
                if hp % 2 == 0:
                    drain_nd(2)
                if hp in (2, 5):
                    drain_q2(1)

            def hook_e1(hp):
                if hp % 2 == 0:
                    drain_nd(2)
                if hp in (1, 4, 7):
                    drain_q2(1)

            for sc in range(2, NCH + 2):
                if sc < NCH:
                    push_nd(sc)
                ctxT_sc = ctxT_tiles.pop(sc - 2)
                emit_accum_eop(ctxT_sc, sc - 2, 0, hook=hook_e0)
                emit_accum_eop(ctxT_sc, sc - 2, 1, hook=hook_e1)
                drain_nd(1)
                drain_q2(1)

    nc.compile()
    return nc


def get_module():
    if "nc" not in _CACHED:
        _CACHED["nc"] = build_module()
    return _CACHED["nc"]


def _bf16(a):
    return np.ascontiguousarray(a).astype(ml_dtypes.bfloat16)


def prepare_in_maps(inputs, Wq, Wk, Wv, Wo, bo):
    """Host-side shard + layout prep. Returns per-core input maps."""
    F8 = ml_dtypes.float8_e4m3
    wq_p = _bf16(np.transpose(np.asarray(Wq), (1, 0, 2)).reshape(D, H * D))
    wkv = np.concatenate([np.asarray(Wk), np.asarray(Wv)], axis=2)  # (H,d,2d)
    wkv_p = _bf16(np.transpose(wkv, (1, 0, 2)).reshape(D, H * 2 * D))
    wo_p = np.transpose(np.asarray(Wo).reshape(H, D, E),
                        (1, 0, 2)).reshape(D, H * E).astype(np.float32) * SW
    wo_hi = wo_p.astype(F8)
    wo_lo = (wo_p - wo_hi.astype(np.float32)).astype(F8)
    in_maps = []
    for b in range(N_CORES):
        xb = np.asarray(inputs[b])
        # xn packed per head: xn[h][p, c*128+j] = x[c*128+p, h*128+j]
        xn_p = _bf16(np.transpose(xb.reshape(NCH, 128, H, D),
                                  (2, 1, 0, 3)).reshape(H, 128, NCH * D))
        in_maps.append({"xT": _bf16(xb.T), "xn": xn_p,
                        "wq": wq_p, "wkv": wkv_p,
                        "wo_hi": wo_hi, "wo_lo": wo_lo})
    return in_maps


def kernel(inputs, Wq, Wk, Wv, Wo, bo):
    B = inputs.shape[0]
    assert B == N_CORES and inputs.shape[1:] == (S, E)
    nc = get_module()
    in_maps = prepare_in_maps(inputs, Wq, Wk, Wv, Wo, bo)
    res = run_bass_kernel_spmd(nc, in_maps, list(range(N_CORES)))
    outs = np.stack([res.results[b]["out"].astype(np.float32)
                     for b in range(N_CORES)], axis=0)
    return (outs + np.asarray(bo, dtype=np.float32)[None, None, :]).astype(
        np.float32)


# revision 81
# speedup vs baseline: 1.0175x; 1.0003x over previous
"""Trainium2 Bass kernel for nn_MultiHeadLinearAttention.

Full-input contract: kernel(**inputs) takes the unsharded numpy inputs and
returns the full output. Internally: data-parallel over batch across the 8
NeuronCores (B == 8, one batch element per core), no collectives.

Per-core math (S=2048, E=2048, H=16, d=128), bf16 matmuls + fp32 PSUM,
with the output projection in fp8e4 DoubleRow (2 k-tiles / 0.5 cyc-per-col):
  Pass A (per head):
    qT  = Wq[h]-stationary route            -> pqT [d, S] (kept for all heads)
    k   = xT-chunk-stationary route         -> pk  [S, d] (natural, transient)
    phi(x) = elu(x)+1 = min(exp(x), 1 + relu(x))   (exact identity)
    Gram trick: G[d', d] = sum_s xh[s,d'] pk[s,d]  (16 accum. matmuls)
                kv = G^T @ Wv[h]  (one matmul; never materializes v)
    ksum[d]  = sum_s pk[s,d]/SC  (ones column scaled by 1/SC)
  Pass B (per s-chunk sc), software-pipelined with the Wo stream:
    per head: nd[s,129] = pqT-chunk^T @ [kv|ksum]; inv = SC/den
    ctx pair: c_hi = fp8(num*inv), c_lo = fp8(num*inv - c_hi), interleaved
    as (hi,lo) byte pairs so one bf16 xbar-transpose moves both.
    out[sc,:] = sum_h sum_t DR-matmul(term_t) * 1/(SC*SW), where the three
    fp8 DoubleRow terms are (c_hi,W_hi), (c_lo,W_hi), (c_hi,W_lo) and
    W_hi+W_lo is the host-side fp8 split of Wo*SW.
Host does: x transpose + bf16/fp8 casts + weight packing + bias add + gather.
"""

import numpy as np
import ml_dtypes

import concourse.bass as bass
import concourse.mybir as mybir
import concourse.tile as tile
from concourse import bacc
from concourse.bass_utils import run_bass_kernel_spmd

S = 2048
E = 2048
H = 16
D = 128
N_CORES = 8
NCH = S // 128  # 16 s-chunks

SC = 32.0   # ctx scale into fp8
SW = 64.0   # Wo scale into fp8
OUT_SCALE = 1.0 / (SC * SW)

F32 = mybir.dt.float32
BF16 = mybir.dt.bfloat16
FP8 = mybir.dt.float8e4
AF = mybir.ActivationFunctionType
ALU = mybir.AluOpType
DR = mybir.MatmulPerfMode.DoubleRow

_CACHED = {}


def build_module():
    nc = bacc.Bacc("TRN2", target_bir_lowering=False, debug=False,
                   num_devices=N_CORES)

    xT = nc.dram_tensor("xT", [E, S], BF16, kind="ExternalInput")
    xn = nc.dram_tensor("xn", [H, 128, NCH * 128], BF16,
                        kind="ExternalInput")
    wq = nc.dram_tensor("wq", [D, H * D], BF16, kind="ExternalInput")
    wkv = nc.dram_tensor("wkv", [D, H * 2 * D], BF16, kind="ExternalInput")
    wo_hi = nc.dram_tensor("wo_hi", [D, H * E], FP8, kind="ExternalInput")
    wo_lo = nc.dram_tensor("wo_lo", [D, H * E], FP8, kind="ExternalInput")
    out = nc.dram_tensor("out", [S, E], BF16, kind="ExternalOutput")

    with tile.TileContext(nc) as tc:
        with (
            tc.tile_pool(name="const", bufs=1) as const,
            tc.tile_pool(name="work", bufs=2) as work,
            tc.tile_pool(name="psum", bufs=2, space="PSUM") as psum,
        ):
            # split the startup DMAs so head 0's slices land first
            wq_sb = const.tile([128, H * D], BF16)
            nc.sync.dma_start(out=wq_sb[:, 0:128], in_=wq[:, 0:128])
            wkv_sb = const.tile([128, H * 2 * D], BF16)
            nc.sync.dma_start(out=wkv_sb[:, 0:256], in_=wkv[:, 0:256])
            ones_col = const.tile([128, 1], BF16)
            nc.vector.memset(ones_col[:], 1.0 / SC)
            warm = const.tile([128, 1], F32)
            nc.vector.memset(warm[:], 0.0)
            nc.scalar.activation(warm[:], warm[:], AF.Exp)
            pqT = const.tile([128, H * S], BF16)      # all heads
            kv_all = const.tile([128, H * 129], BF16)  # all heads [kv|ksum]

            ctxT_tiles = {}
            ctx_tiles = {}

            def emit_nd_chunk(sc, h):
                ctx_sc = ctx_tiles[sc]
                pair8 = ctx_sc[:].bitcast(FP8).rearrange(
                    "p hh (j two) -> p hh j two", two=2)
                ndp = psum.tile([128, 129], F32, tag="g", bufs=3)
                nc.tensor.matmul(
                    ndp[:],
                    pqT[:, h * S + sc * 128:h * S + (sc + 1) * 128],
                    kv_all[:, h * 129:(h + 1) * 129],
                    start=True, stop=True)
                inv = work.tile([128, 1], F32, tag="inv", bufs=8)
                nc.vector.reciprocal(inv[:], ndp[:, 128:129])
                nc.scalar.activation(pair8[:, h, :, 0], ndp[:, 0:128],
                                     AF.Copy, scale=inv[:, 0:1])
                nc.vector.scalar_tensor_tensor(
                    pair8[:, h, :, 1], ndp[:, 0:128], inv[:, 0:1],
                    pair8[:, h, :, 0], ALU.mult, ALU.subtract)

            xh2_pending = []
            q2_done = set()

            # deferred q pieces: (piece_idx, col_start, col_len)
            Q2_PIECES = [(0, 256, 256), (1, 512, 512),
                         (2, 1024, 512), (3, 1536, 512)]
            Q2_ORDER = (1, 2, 3)

            def prefetch_q2(h, piece):
                _, c0, cl = Q2_PIECES[piece]
                xh2 = work.tile([128, 512], BF16, tag="xh2", bufs=4)
                nc.sync.dma_start(
                    out=xh2[:, 0:cl],
                    in_=xT[h * 128:(h + 1) * 128, c0:c0 + cl])
                xh2_pending.append(xh2)

            def emit_q2_half(h, piece):
                _, c0, cl = Q2_PIECES[piece]
                xh2 = xh2_pending.pop(0)
                q2p = psum.tile([128, 512], F32, tag="kv", bufs=1)
                nc.tensor.matmul(
                    q2p[:, 0:cl], wq_sb[:, h * 128:(h + 1) * 128],
                    xh2[:, 0:cl], start=True, stop=True)
                base = h * S + c0
                e2 = work.tile([128, 512], BF16, tag="e", bufs=3)
                nc.scalar.activation(e2[:, 0:cl], q2p[:, 0:cl], AF.Exp)
                t2 = work.tile([128, 512], BF16, tag="t", bufs=2)
                nc.vector.tensor_scalar(t2[:, 0:cl], q2p[:, 0:cl], 0.0, 1.0,
                                        ALU.max, ALU.add)
                nc.vector.tensor_tensor(pqT[:, base:base + cl],
                                        e2[:, 0:cl], t2[:, 0:cl], ALU.min)

            def emit_transpose(sc):
                ctx_sc = ctx_tiles.pop(sc)
                ctxT_sc = work.tile([128, H, 128], BF16, tag="ctxT", bufs=2)
                nc.sync.dma_start(
                    out=ctxT_sc[:],
                    in_=ctx_sc[:].rearrange("p h j -> p (h j)"),
                    transpose=True)
                ctxT_tiles[sc] = ctxT_sc

            # ---- pass-B step queue: nd heads + transpose, interleaved under
            # the PE-bound Wo stream so the scale chains never block PE.
            nd_steps = []

            def push_nd(sc):
                ctx_sc = work.tile([128, H, 128], BF16, tag="ctx", bufs=2)
                ctx_tiles[sc] = ctx_sc
                for h in range(H):
                    nd_steps.append((sc, h))
                nd_steps.append((sc, -1))  # transpose marker

            def drain_nd(n):
                for _ in range(n):
                    if not nd_steps:
                        return
                    sc, h = nd_steps.pop(0)
                    if h < 0:
                        emit_transpose(sc)
                    else:
                        # the nd read of pqT cols sc*128:(sc+1)*128 must be
                        # emitted after the deferred q2 write of that piece
                        while sc >= 4 and (h, sc // 4) not in q2_done:
                            drain_q2(1)
                        emit_nd_chunk(sc, h)

            def emit_accum_eop(ctxT_sc, sc, eop, hook=None):
                cT8 = ctxT_sc[:].bitcast(FP8).rearrange(
                    "p h (j two) -> p h j two", two=2)
                pa = psum.tile([128, 1024], F32, tag="pj", bufs=2)
                for hp in range(8):
                    if hook is not None:
                        hook(hp)
                    lhi = cT8[:, 2 * hp:2 * hp + 2, :, 0]
                    llo = cT8[:, 2 * hp:2 * hp + 2, :, 1]
                    for half in range(2):
                        eoq = eop * 2 + half
                        rhi = woq_hi[eoq][:, 2 * hp:2 * hp + 2, :]
                        rlo = woq_lo[eoq][:, 2 * hp:2 * hp + 2, :]
                        ps = pa[:, half * 512:(half + 1) * 512]
                        first = hp == 0
                        last = hp == 7
                        nc.tensor.matmul(ps, lhi, rhi, start=first,
                                         stop=False, perf_mode=DR)
                        nc.tensor.matmul(ps, llo, rhi, start=False,
                                         stop=False, perf_mode=DR)
                        nc.tensor.matmul(ps, lhi, rlo, start=False,
                                         stop=last, perf_mode=DR)
                if eop == 0:
                    out_t = work.tile([128, 2048], BF16, tag="outsb", bufs=2)
                    out_tiles[sc] = out_t
                    nc.scalar.activation(out_t[:, 0:1024], pa[:], AF.Copy,
                                         scale=OUT_SCALE)
                    if sc >= NCH - 2:
                        nc.sync.dma_start(
                            out=out[sc * 128:(sc + 1) * 128, 0:1024],
                            in_=out_t[:, 0:1024])
                else:
                    out_t = out_tiles.pop(sc)
                    if sc >= NCH - 2:
                        # split scale+store at 512 granularity across BOTH
                        # engines so the last stores run in parallel
                        for hh in range(2):
                            c0 = 1024 + hh * 512
                            ps = pa[:, hh * 512:(hh + 1) * 512]
                            if hh == 0:
                                nc.scalar.activation(
                                    out_t[:, c0:c0 + 512], ps, AF.Copy,
                                    scale=OUT_SCALE)
                            else:
                                nc.vector.tensor_scalar(
                                    out_t[:, c0:c0 + 512], ps,
                                    OUT_SCALE, None, ALU.mult)
                            nc.sync.dma_start(
                                out=out[sc * 128:(sc + 1) * 128,
                                        c0:c0 + 512],
                                in_=out_t[:, c0:c0 + 512])
                    else:
                        nc.scalar.activation(out_t[:, 1024:2048], pa[:],
                                             AF.Copy, scale=OUT_SCALE)
                        nc.sync.dma_start(
                            out=out[sc * 128:(sc + 1) * 128, :],
                            in_=out_t[:])

            wo_hi_v = wo_hi[:].rearrange("p (h q x) -> p h q x", q=4, x=512)
            wo_lo_v = wo_lo[:].rearrange("p (h q x) -> p h q x", q=4, x=512)
            woq_hi = []
            woq_lo = []
            out_tiles = {}

            # -------- Pass A: q/k projections, phi, Gram kv, per head -----
            xhT_pending = {}

            def prefetch_xhT(h):
                xhT = work.tile([128, S], BF16, tag="xhT", bufs=2)
                if h == 0:
                    nc.sync.dma_start(out=xhT[:, 0:512],
                                      in_=xT[0:128, 0:512])
                    nc.sync.dma_start(out=xhT[:, 512:2048],
                                      in_=xT[0:128, 512:2048])
                else:
                    nc.sync.dma_start(out=xhT[:],
                                      in_=xT[h * 128:(h + 1) * 128, :])
                xhT_pending[h] = xhT

            xn_pending = {}

            def prefetch_xn(h):
                xn_sb = work.tile([128, NCH, 128], BF16, tag="xn", bufs=2)
                # flat APs on both sides: 4KB contiguous descriptors (a 3-D
                # out AP forces 256B descriptors and doubles transfer time)
                nc.sync.dma_start(
                    out=xn_sb[:].rearrange("p c j -> p (c j)"),
                    in_=xn[h])
                xn_pending[h] = xn_sb

            prefetch_xhT(0)
            prefetch_xhT(1)
            for h in range(H):
                xhT = xhT_pending.pop(h)
                if h + 2 < H:
                    prefetch_xhT(h + 2)
                prefetch_xn(h)
                xn_sb = xn_pending.pop(h)
                if h == 0:
                    nc.sync.dma_start(out=wq_sb[:, 128:512],
                                      in_=wq[:, 128:512])
                    nc.sync.dma_start(out=wkv_sb[:, 256:1024],
                                      in_=wkv[:, 256:1024])
                elif h == 1:
                    nc.sync.dma_start(out=wq_sb[:, 512:1024],
                                      in_=wq[:, 512:1024])
                    nc.sync.dma_start(out=wkv_sb[:, 1024:2048],
                                      in_=wkv[:, 1024:2048])
                elif h == 2:
                    nc.sync.dma_start(out=wq_sb[:, 1024:],
                                      in_=wq[:, 1024:])
                    nc.sync.dma_start(out=wkv_sb[:, 2048:3072],
                                      in_=wkv[:, 2048:3072])
                elif h == 4:
                    nc.sync.dma_start(out=wkv_sb[:, 3072:],
                                      in_=wkv[:, 3072:])

                # q (transposed layout) + phi -> pqT[h, 0:512]; the other
                # three 512-blocks are deferred into pass B (emit_q2_half)
                # to overlap their DVE work under the PE-bound Wo stream
                qp = psum.tile([128, 1024], F32, tag="pj", bufs=2)
                nc.tensor.matmul(
                    qp[:, 0:512], wq_sb[:, h * 128:(h + 1) * 128],
                    xhT[:, 0:512], start=True, stop=True)
                eq = work.tile([128, 512], BF16, tag="e", bufs=3)
                nc.scalar.activation(eq[:], qp[:, 0:512], AF.Exp)
                tq = work.tile([128, 512], BF16, tag="t", bufs=2)
                if h % 2 == 1:
                    nc.scalar.activation(tq[:], qp[:, 0:512], AF.Relu)
                    nc.vector.scalar_tensor_tensor(
                        pqT[:, h * S:h * S + 512], tq[:], 1.0, eq[:],
                        ALU.add, ALU.min)
                else:
                    nc.vector.tensor_scalar(tq[:], qp[:, 0:512], 0.0, 1.0,
                                            ALU.max, ALU.add)
                    nc.vector.tensor_tensor(pqT[:, h * S:h * S + 512],
                                            eq[:], tq[:], ALU.min)

                # k (natural layout) + phi -> pk
                # 'act' variant (exp + relu on ACT, fused add/min on DVE) vs
                # 'dve' variant (exp on ACT, add + min on DVE): alternate by
                # head parity so adjacent heads occupy different engines and
                # pipeline across the head loop.
                pk = work.tile([128, S], BF16, tag="pk", bufs=2)
                for j in range(2):
                    kp = psum.tile([128, 1024], F32, tag="pj", bufs=2)
                    for c in range(8):
                        sc = j * 8 + c
                        nc.tensor.matmul(
                            kp[:, c * 128:(c + 1) * 128],
                            xhT[:, sc * 128:(sc + 1) * 128],
                            wkv_sb[:, h * 256:h * 256 + 128],
                            start=True, stop=True)
                    dst = pk[:, j * 1024:(j + 1) * 1024]
                    e = work.tile([128, 1024], BF16, tag="e", bufs=3)
                    nc.scalar.activation(e[:], kp[:], AF.Exp)
                    if j == h % 2:
                        r = work.tile([128, 1024], BF16, tag="t", bufs=2)
                        nc.scalar.activation(r[:], kp[:], AF.Relu)
                        nc.vector.scalar_tensor_tensor(
                            dst, r[:], 1.0, e[:], ALU.add, ALU.min)
                    else:
                        t = work.tile([128, 1024], BF16, tag="t", bufs=2)
                        nc.vector.tensor_scalar(t[:], kp[:], 0.0, 1.0,
                                                ALU.max, ALU.add)
                        nc.vector.tensor_tensor(dst, e[:], t[:], ALU.min)

                if h == 10:
                    for eoq in range(4):
                        whi = work.tile([128, H, 512], FP8,
                                        tag=f"whi{eoq}", bufs=1)
                        nc.sync.dma_start(out=whi[:], in_=wo_hi_v[:, :, eoq, :])
                        woq_hi.append(whi)
                        wlo = work.tile([128, H, 512], FP8,
                                        tag=f"wlo{eoq}", bufs=1)
                        nc.sync.dma_start(out=wlo[:], in_=wo_lo_v[:, :, eoq, :])
                        woq_lo.append(wlo)

                # Gram G[d', d] + ksum -> kv_all[h]
                gp = psum.tile([128, 128], F32, tag="g", bufs=3)
                kvp = psum.tile([128, 129], F32, tag="kv", bufs=1)
                for c in range(NCH):
                    nc.tensor.matmul(gp[:], xn_sb[:, c, :],
                                     pk[:, c * 128:(c + 1) * 128],
                                     start=(c == 0), stop=(c == NCH - 1))
                    nc.tensor.matmul(kvp[:, 128:129],
                                     pk[:, c * 128:(c + 1) * 128],
                                     ones_col[:],
                                     start=(c == 0), stop=(c == NCH - 1))
                g_sb = work.tile([128, 128], BF16, tag="gsb")
                nc.vector.tensor_copy(g_sb[:], gp[:])
                nc.tensor.matmul(kvp[:, 0:128], g_sb[:],
                                 wkv_sb[:, h * 256 + 128:h * 256 + 256],
                                 start=True, stop=True)
                nc.vector.tensor_copy(
                    kv_all[:, h * 129:(h + 1) * 129], kvp[:])
                if h == 0:
                    for esc in range(2):
                        ctxe = work.tile([128, H, 128], BF16,
                                         tag="ctx", bufs=2)
                        ctx_tiles[esc] = ctxe
                for esc in range(2):
                    emit_nd_chunk(esc, h)

            emit_transpose(0)
            emit_transpose(1)
            q2q = [(h, p) for p in Q2_ORDER for h in range(H)]
            q2q.reverse()  # pop() from the front

            q2pf = list(q2q)

            def drain_q2(n):
                for _ in range(n):
                    while q2pf and len(xh2_pending) < 4:
                        hq = q2pf.pop()
                        prefetch_q2(*hq)
                    if q2q:
                        h, half = q2q.pop()
                        emit_q2_half(h, half)
                        q2_done.add((h, half))

            def hook_e0(hp):
                # 2 nd steps every other hp: 16 heads + transpose over both eops
                if hp % 2 == 0:
                    drain_nd(2)
                if hp in (2, 5):
                    drain_q2(1)

            def hook_e1(hp):
                if hp % 2 == 0:
                    drain_nd(2)
                if hp in (1, 4, 7):
                    drain_q2(1)

            for sc in range(2, NCH + 2):
                if sc < NCH:
                    push_nd(sc)
                ctxT_sc = ctxT_tiles.pop(sc - 2)
                emit_accum_eop(ctxT_sc, sc - 2, 0, hook=hook_e0)
                emit_accum_eop(ctxT_sc, sc - 2, 1, hook=hook_e1)
                drain_nd(1)
                drain_q2(1)

    nc.compile()
    return nc


def get_module():
    if "nc" not in _CACHED:
        _CACHED["nc"] = build_module()
    return _CACHED["nc"]


def _bf16(a):
    return np.ascontiguousarray(a).astype(ml_dtypes.bfloat16)


def prepare_in_maps(inputs, Wq, Wk, Wv, Wo, bo):
    """Host-side shard + layout prep. Returns per-core input maps."""
    F8 = ml_dtypes.float8_e4m3
    wq_p = _bf16(np.transpose(np.asarray(Wq), (1, 0, 2)).reshape(D, H * D))
    wkv = np.concatenate([np.asarray(Wk), np.asarray(Wv)], axis=2)  # (H,d,2d)
    wkv_p = _bf16(np.transpose(wkv, (1, 0, 2)).reshape(D, H * 2 * D))
    wo_p = np.transpose(np.asarray(Wo).reshape(H, D, E),
                        (1, 0, 2)).reshape(D, H * E).astype(np.float32) * SW
    wo_hi = wo_p.astype(F8)
    wo_lo = (wo_p - wo_hi.astype(np.float32)).astype(F8)
    in_maps = []
    for b in range(N_CORES):
        xb = np.asarray(inputs[b])
        # xn packed per head: xn[h][p, c*128+j] = x[c*128+p, h*128+j]
        xn_p = _bf16(np.transpose(xb.reshape(NCH, 128, H, D),
                                  (2, 1, 0, 3)).reshape(H, 128, NCH * D))
        in_maps.append({"xT": _bf16(xb.T), "xn": xn_p,
                        "wq": wq_p, "wkv": wkv_p,
                        "wo_hi": wo_hi, "wo_lo": wo_lo})
    return in_maps


def kernel(inputs, Wq, Wk, Wv, Wo, bo):
    B = inputs.shape[0]
    assert B == N_CORES and inputs.shape[1:] == (S, E)
    nc = get_module()
    in_maps = prepare_in_maps(inputs, Wq, Wk, Wv, Wo, bo)
    res = run_bass_kernel_spmd(nc, in_maps, list(range(N_CORES)))
    outs = np.stack([res.results[b]["out"].astype(np.float32)
                     for b in range(N_CORES)], axis=0)
    return (outs + np.asarray(bo, dtype=np.float32)[None, None, :]).astype(
        np.float32)
